# revision 19
# baseline (speedup 1.0000x reference)
"""Trainium2 Bass kernel for nn_BackbonePointNet (3-layer PointNet-style GNN).

Sharding: destination nodes across 8 cores (12.5K nodes / 200K edges each).
Per layer l the edge MLP factors as
    pre(e) = u_l[src_e] + v_l[dst_e]
    msg(e) = relu(pre) @ wb_l
    h(i)   = relu(max_{e->i} msg(e) + bb_l)
with u_l = h_{l-1} @ w_lh + pos @ w_lp computed per-core for local nodes
then AllGather-replicated in bf16, and v_l = b_la - pos_i @ w_lp (layer-a
bias folded into v) precomputed on host.

The indirect-DMA gather path supports one offset per partition (128 rows
per op, ~1us SWDGE overhead each on the Pool engine), which makes Pool the
bottleneck engine: ~1568 gather ops per layer per core.  Everything else
is structured to hide under the Pool stream:
  * layers 1/2 (64-wide) process PAIRS of 64-node tiles with A-features on
    partitions 0:64 and B-features on 64:128; block-diagonal weights let
    one 512-row PE stream cover 1024 edges (v-selector + second linear);
  * the per-node u matmuls for the next layer run per-pair right after
    each pair's segment-max, and the u AllGather is split into 7 chunks
    launched as their node ranges complete, overlapping the remaining
    gather stream (collectives occupy their own resource, not Pool);
  * u tables are stored in chunk-major order (chunk, core, row) so each
    AllGather chunk is contiguous; gather indices are relabeled on host.
Pooling (segment mean, sorted batch) and the 2-layer regressor + sigmoid
run on host.
"""

import numpy as np
import ml_dtypes

N = 100_000
E = 16 * N
B = 64
NCORES = 8
NCHUNK = 7

_BF = ml_dtypes.bfloat16
_CACHE = {}


# --------------------------------------------------------------------------
# device program
# --------------------------------------------------------------------------

def _build_nc(n_nodes, n_loc_pad, d_grp, n_cores, collectives=True, layers=3):
    from concourse import bass, mybir, tile  # noqa: F401
    import concourse.bacc as bacc

    BF16 = mybir.dt.bfloat16
    F32 = mybir.dt.float32
    I32 = mybir.dt.int32
    AF = mybir.ActivationFunctionType

    npt2 = 1024 // d_grp            # nodes per pair-half (64 for d_grp=16)
    rows_pp = 2 * npt2              # nodes per pair-tile
    n_loc = n_nodes // n_cores      # real (unpadded) local nodes
    npair = n_loc_pad // rows_pp     # pair-tiles for layers 1/2
    nt3 = n_loc_pad // npt2          # tiles for layer 3
    ROWS_OP = 4096                   # gathered rows per gt staging tile
    cpo12 = ROWS_OP // 128           # gather ops per gt tile (32)
    ppo = cpo12 // 16                # pairs per gt tile (2)
    tpo = cpo12 // 8                 # layer-3 tiles per gt tile (4)
    nop12 = npair // ppo
    nop3 = nt3 // tpo
    assert npair % ppo == 0 and nt3 % tpo == 0
    pb = _chunk_pair_bounds(npair)   # tapered collective chunk boundaries
    rb = [b * rows_pp for b in pb]   # row boundaries per core
    n_tab = n_cores * n_loc_pad      # chunk-major replicated table rows

    nc = bacc.Bacc("TRN2", target_bir_lowering=False, debug=False,
                   num_devices=n_cores)

    # ---- external inputs (shared across cores) ----
    u1_full = nc.dram_tensor("u1_full", [n_tab, 64], BF16, kind="ExternalInput")
    selAB = nc.dram_tensor("selAB", [128, 1024], BF16, kind="ExternalInput")
    id128 = nc.dram_tensor("id128", [128, 128], BF16, kind="ExternalInput")
    wbd1 = nc.dram_tensor("wbd1", [128, 128], BF16, kind="ExternalInput")
    wbd2 = nc.dram_tensor("wbd2", [128, 128], BF16, kind="ExternalInput")
    w3b = nc.dram_tensor("w3b", [128, 128], BF16, kind="ExternalInput")
    wst2 = nc.dram_tensor("wst2", [128, 64], BF16, kind="ExternalInput")
    wst3 = nc.dram_tensor("wst3", [128, 128], BF16, kind="ExternalInput")
    wpst2 = nc.dram_tensor("wpst2", [35, 64], BF16, kind="ExternalInput")
    wpst3 = nc.dram_tensor("wpst3", [35, 128], BF16, kind="ExternalInput")
    fbias = nc.dram_tensor("fbias", [128, 3], F32, kind="ExternalInput")
    # ---- external inputs (per-core) ----
    gidx12 = nc.dram_tensor("gidx12", [128, npair * 16], I32, kind="ExternalInput")
    gidx3 = nc.dram_tensor("gidx3", [128, nt3 * 8], I32, kind="ExternalInput")
    vp1 = nc.dram_tensor("vp1", [128, npair * 128], BF16, kind="ExternalInput")
    vp2 = nc.dram_tensor("vp2", [128, npair * 128], BF16, kind="ExternalInput")
    vp3 = nc.dram_tensor("vp3", [128, (nt3 // 2) * 128], BF16, kind="ExternalInput")
    posTp = nc.dram_tensor("posTp", [35, npair * npt2], BF16, kind="ExternalInput")
    hT3_out = nc.dram_tensor("hT3", [128, n_loc_pad], F32, kind="ExternalOutput")

    # internal dram for u slices / replicated chunk-major tables
    u2_slice = nc.dram_tensor("u2_slice", [n_loc_pad, 64], BF16, kind="Internal")
    u3_slice = nc.dram_tensor("u3_slice", [n_loc_pad, 128], BF16, kind="Internal")
    u2_full = nc.dram_tensor("u2_full", [n_tab, 64], BF16, kind="Internal",
                             addr_space="Shared")
    u3_full = nc.dram_tensor("u3_full", [n_tab, 128], BF16, kind="Internal",
                             addr_space="Shared")

    with tile.TileContext(nc) as tc:
        with tc.tile_pool(name="const", bufs=1) as cp, \
             tc.tile_pool(name="gath", bufs=3) as gp, \
             tc.tile_pool(name="work", bufs=3) as wp, \
             tc.tile_pool(name="out", bufs=2) as op, \
             tc.tile_pool(name="hbuf", bufs=1) as hp, \
             tc.tile_pool(name="psum", bufs=2, space="PSUM") as pp:

            # ---- resident constants ----
            def load_const(handle, shape, dt):
                t = cp.tile(shape, dt, tag=handle.name)
                nc.sync.dma_start(out=t[:], in_=handle[:])
                return t

            gidx12_t = load_const(gidx12, [128, npair * 16], I32)
            gidx3_t = load_const(gidx3, [128, nt3 * 8], I32)
            sel_t = load_const(selAB, [128, 1024], BF16)
            id_t = load_const(id128, [128, 128], BF16)
            wbd1_t = load_const(wbd1, [128, 128], BF16)
            wbd2_t = load_const(wbd2, [128, 128], BF16)
            w3b_t = load_const(w3b, [128, 128], BF16)
            wst2_t = load_const(wst2, [128, 64], BF16)
            wst3_t = load_const(wst3, [128, 128], BF16)
            wpst2_t = load_const(wpst2, [35, 64], BF16)
            wpst3_t = load_const(wpst3, [35, 128], BF16)
            fb_t = load_const(fbias, [128, 3], F32)
            vp1_t = load_const(vp1, [128, npair * 128], BF16)
            vp2_t = load_const(vp2, [128, npair * 128], BF16)
            vp3_t = load_const(vp3, [128, (nt3 // 2) * 128], BF16)
            posTp_t = load_const(posTp, [35, npair * npt2], BF16)

            hT1 = hp.tile([128, npair * npt2], BF16, tag="hT1")
            hT2 = hp.tile([128, npair * npt2], BF16, tag="hT2")

            # ---- per-pair u matmuls for the next layer's table ------------
            def u_pair(hT, wst_t, wpst_t, c_out, u_slice, pt, state):
                nbat = 2
                q = pt % nbat
                if q == 0:
                    us4 = op.tile([128, nbat * c_out], BF16, tag="us4")
                    state["us"] = us4
                us4 = state["us"]
                ps = pp.tile([128, 1024], F32, tag="pre", space="PSUM")
                nc.tensor.matmul(
                    out=ps[0:64, :c_out],
                    lhsT=hT[0:64, pt * npt2:(pt + 1) * npt2],
                    rhs=wst_t[0:64, :], start=True, stop=False)
                nc.tensor.matmul(
                    out=ps[0:64, :c_out],
                    lhsT=posTp_t[0:3, pt * npt2:(pt + 1) * npt2],
                    rhs=wpst_t[0:3, :], start=False, stop=True,
                    skip_group_check=True)
                nc.tensor.matmul(
                    out=ps[64:128, :c_out],
                    lhsT=hT[64:128, pt * npt2:(pt + 1) * npt2],
                    rhs=wst_t[64:128, :], start=True, stop=False,
                    skip_group_check=True)
                nc.tensor.matmul(
                    out=ps[64:128, :c_out],
                    lhsT=posTp_t[32:35, pt * npt2:(pt + 1) * npt2],
                    rhs=wpst_t[32:35, :], start=False, stop=True,
                    skip_group_check=True)
                nc.scalar.activation(out=us4[:, q * c_out:(q + 1) * c_out],
                                     in_=ps[:, :c_out], func=AF.Copy)
                if q == nbat - 1:
                    m0 = pt - q
                    lo = m0 * rows_pp
                    hi = (pt + 1) * rows_pp
                    if rows_pp == 128:
                        nc.sync.dma_start(
                            out=u_slice[lo:hi, :].rearrange(
                                "(b p) f -> p b f", p=128),
                            in_=us4[:])
                    else:
                        for j in range(nbat):
                            l2 = (m0 + j) * rows_pp
                            for half in range(2):
                                l3 = l2 + half * npt2
                                nc.sync.dma_start(
                                    out=u_slice[l3:l3 + npt2, :],
                                    in_=us4[half * 64:half * 64 + npt2,
                                            j * c_out:(j + 1) * c_out])

            def coll_chunk(u_slice, u_full, c_out, g):
                lo, hi = rb[g], rb[g + 1]
                base = n_cores * lo
                if collectives:
                    nc.gpsimd.collective_compute(
                        "AllGather", mybir.AluOpType.bypass,
                        replica_groups=[list(range(n_cores))],
                        ins=[u_slice[lo:hi, :]],
                        outs=[u_full[base:base + n_cores * (hi - lo), :]])
                else:
                    nc.sync.dma_start(
                        out=u_full[base:base + hi - lo, :],
                        in_=u_slice[lo:hi, :])

            # ------------- paired edge phase (layers 1/2, 64-wide) --------
            # One indirect-DMA op per 128 gathered rows (the only offset
            # shape the DGE path supports).
            def edge_phase12(u_ap, vp_t, wbd_t, hT, bcol, nxt):
                state = {}
                for g in range(nop12):
                    gt = gp.tile([128, ROWS_OP], BF16, tag="g")
                    for col in range(cpo12):
                        # skip chunks whose 128 edges all belong to padding
                        # nodes (>= n_loc); their columns are never read by
                        # the host and stale staging data is finite
                        pt_c = g * ppo + col // 16
                        half_c = (col % 16) // 8
                        node0 = pt_c * 2 * npt2 + half_c * npt2 +                             (col % 8) * (npt2 // 8)
                        if node0 >= n_loc:
                            continue
                        nc.gpsimd.indirect_dma_start(
                            out=gt[:, col * 64:(col + 1) * 64], out_offset=None,
                            in_=u_ap,
                            in_offset=bass.IndirectOffsetOnAxis(
                                ap=gidx12_t[:, g * cpo12 + col:
                                            g * cpo12 + col + 1], axis=0),
                        )
                    for pl in range(ppo):
                        pt = g * ppo + pl
                        pre = pp.tile([128, 1024], F32, tag="pre", space="PSUM")
                        for h in range(2):
                            nc.tensor.matmul(
                                out=pre[:, h * 512:(h + 1) * 512],
                                lhsT=vp_t[:, pt * 128:(pt + 1) * 128],
                                rhs=sel_t[:, h * 512:(h + 1) * 512],
                                start=True, stop=False)
                        for half in range(2):  # 0=A rows 0:64, 1=B rows 64:128
                            r0 = half * 64
                            for c in range(8):
                                cc = pl * 16 + half * 8 + c
                                nc.tensor.matmul(
                                    out=pre[r0:r0 + 64, c * 128:(c + 1) * 128],
                                    lhsT=gt[:, cc * 64:(cc + 1) * 64],
                                    rhs=id_t[:],
                                    start=False, stop=(c % 4 == 3),
                                    skip_group_check=True)
                        prs = wp.tile([128, 1024], BF16, tag="prs")
                        nc.scalar.activation(out=prs[:], in_=pre[:], func=AF.Relu)
                        msg = pp.tile([128, 1024], F32, tag="msg", space="PSUM")
                        for h in range(2):
                            nc.tensor.matmul(
                                out=msg[:, h * 512:(h + 1) * 512],
                                lhsT=wbd_t[:],
                                rhs=prs[:, h * 512:(h + 1) * 512],
                                start=True, stop=True)
                        nc.vector.tensor_reduce(
                            out=hT[:, pt * npt2:(pt + 1) * npt2],
                            in_=msg[:].rearrange("p (n k) -> p n k", k=d_grp),
                            axis=mybir.AxisListType.X, op=mybir.AluOpType.max)
                        # bias+relu this pair's columns, then its u matmuls
                        nc.scalar.activation(
                            out=hT[:, pt * npt2:(pt + 1) * npt2],
                            in_=hT[:, pt * npt2:(pt + 1) * npt2],
                            func=AF.Relu, bias=fb_t[:, bcol:bcol + 1],
                            scale=1.0)
                        if nxt is not None:
                            u_pair(hT, nxt["wst"], nxt["wpst"], nxt["c_out"],
                                   nxt["slice"], pt, state)
                            if pt + 1 in pb:
                                coll_chunk(nxt["slice"], nxt["full"],
                                           nxt["c_out"],
                                           pb.index(pt + 1) - 1)

            # ------------- layer-3 edge phase (128-wide, unpaired) --------
            def edge_phase3(u_ap):
                nbat = 4
                ho4 = None
                t0 = 0
                for g in range(nop3):
                    gt = gp.tile([128, ROWS_OP], BF16, tag="g")
                    for col in range(cpo12):
                        t_c = g * tpo + col // 8
                        node0 = t_c * npt2 + (col % 8) * (npt2 // 8)
                        if node0 >= n_loc:
                            continue
                        nc.gpsimd.indirect_dma_start(
                            out=gt[:, col * 128:(col + 1) * 128], out_offset=None,
                            in_=u_ap,
                            in_offset=bass.IndirectOffsetOnAxis(
                                ap=gidx3_t[:, g * cpo12 + col:
                                           g * cpo12 + col + 1], axis=0),
                        )
                    for tl in range(tpo):
                        t = g * tpo + tl
                        r0 = (t % 2) * 64
                        pre = pp.tile([128, 1024], F32, tag="pre", space="PSUM")
                        for h in range(2):
                            nc.tensor.matmul(
                                out=pre[:, h * 512:(h + 1) * 512],
                                lhsT=vp3_t[r0:r0 + npt2,
                                           (t // 2) * 128:(t // 2) * 128 + 128],
                                rhs=sel_t[r0:r0 + npt2, h * 512:(h + 1) * 512],
                                start=True, stop=False)
                        for c in range(8):
                            cc = tl * 8 + c
                            nc.tensor.matmul(
                                out=pre[:, c * 128:(c + 1) * 128],
                                lhsT=gt[:, cc * 128:(cc + 1) * 128],
                                rhs=id_t[:],
                                start=False, stop=(c % 4 == 3),
                                skip_group_check=True)
                        prs = wp.tile([128, 1024], BF16, tag="prs")
                        nc.scalar.activation(out=prs[:], in_=pre[:], func=AF.Relu)
                        msg = pp.tile([128, 1024], F32, tag="msg", space="PSUM")
                        for h in range(2):
                            nc.tensor.matmul(
                                out=msg[:, h * 512:(h + 1) * 512],
                                lhsT=w3b_t[:],
                                rhs=prs[:, h * 512:(h + 1) * 512],
                                start=True, stop=True)
                        q = t % nbat
                        if q == 0:
                            ho4 = op.tile([128, nbat * npt2], F32, tag="ho4")
                            t0 = t
                        nc.vector.tensor_reduce(
                            out=ho4[:, q * npt2:(q + 1) * npt2],
                            in_=msg[:].rearrange("p (n k) -> p n k", k=d_grp),
                            axis=mybir.AxisListType.X, op=mybir.AluOpType.max)
                        if q == nbat - 1 or t == nt3 - 1:
                            nb = q + 1
                            hf = op.tile([128, nbat * npt2], F32, tag="hf")
                            nc.scalar.activation(
                                out=hf[:, :nb * npt2], in_=ho4[:, :nb * npt2],
                                func=AF.Relu, bias=fb_t[:, 2:3], scale=1.0)
                            nc.sync.dma_start(
                                out=hT3_out[:, t0 * npt2:t0 * npt2 + nb * npt2],
                                in_=hf[:, :nb * npt2])

            def debug_dump(hT):
                for s in range(4):
                    w4 = npair * npt2 // 4
                    hf = op.tile([128, w4], F32, tag="hfd")
                    nc.scalar.activation(out=hf[:], in_=hT[:, s * w4:(s + 1) * w4],
                                         func=AF.Copy)
                    nc.sync.dma_start(out=hT3_out[:, s * w4:(s + 1) * w4],
                                      in_=hf[:])

            nxt2 = dict(wst=wst2_t, wpst=wpst2_t, c_out=64,
                        slice=u2_slice, full=u2_full)
            nxt3 = dict(wst=wst3_t, wpst=wpst3_t, c_out=128,
                        slice=u3_slice, full=u3_full)

            # ---------------- layer 1 ----------------
            edge_phase12(u1_full[:], vp1_t, wbd1_t, hT1, 0,
                         nxt2 if layers >= 2 else None)
            if layers >= 2:
                # ---------------- layer 2 ----------------
                edge_phase12(u2_full[:], vp2_t, wbd2_t, hT2, 1,
                             nxt3 if layers >= 3 else None)
            if layers >= 3:
                # ---------------- layer 3 ----------------
                edge_phase3(u3_full[:])
            elif layers == 1:
                debug_dump(hT1)
            else:
                debug_dump(hT2)

    nc.compile()
    return nc


# --------------------------------------------------------------------------
# host side
# --------------------------------------------------------------------------

def _chunk_pair_bounds(npair):
    # tapered chunk sizes (halving, even, min 2) so the final collective
    # chunk -- the only one not hidden behind the gather stream -- is small
    if npair < 8 or npair % 2:
        return [0, npair]
    sizes = []
    rem = npair
    while rem > 2:
        s = max(2, -(-rem // 2) // 2 * 2 + (2 if (-(-rem // 2)) % 2 else 0))
        s = min(s, rem - 2)
        sizes.append(s)
        rem -= s
    sizes.append(rem)
    return [0] + list(np.cumsum(sizes))


def _next_pow2_ge(x, lo=16):
    d = lo
    while d < x:
        d *= 2
    return d


def _posw_stack(wp):
    out = np.zeros((35, wp.shape[1]), np.float32)
    out[0:3] = wp
    out[32:35] = wp
    return out.astype(_BF)


def _blockdiag2(w):
    c = w.shape[0]
    out = np.zeros((128, 2 * w.shape[1]), np.float32)
    out[:c, :w.shape[1]] = w
    out[64:64 + c, w.shape[1]:] = w
    return out


def _prep(pos, edge_index, weights, n_cores):
    n_nodes = pos.shape[0]
    src = edge_index[0].astype(np.int64)
    dst = edge_index[1].astype(np.int64)
    e_tot = src.shape[0]

    canonical = (e_tot == 16 * n_nodes) and np.array_equal(
        dst, np.repeat(np.arange(n_nodes, dtype=np.int64), e_tot // n_nodes))

    if canonical and e_tot // n_nodes == 16:
        d_grp = 16
        slot_src = src.reshape(n_nodes, 16)
        deg0 = None
    else:
        order = np.argsort(dst, kind="stable")
        s_sorted = src[order]
        counts = np.bincount(dst, minlength=n_nodes)
        d_grp = _next_pow2_ge(int(counts.max()) if e_tot else 16)
        starts = np.concatenate([[0], np.cumsum(counts)])
        slot_src = np.zeros((n_nodes, d_grp), np.int64)
        idx = np.arange(d_grp)
        for i in range(n_nodes):
            c = counts[i]
            if c:
                row = s_sorted[starts[i]:starts[i] + c]
                slot_src[i] = row[idx % c]
        deg0 = counts == 0

    n_loc = n_nodes // n_cores
    npt2 = 1024 // d_grp
    rows_pp = 2 * npt2
    n_loc_pad = int(np.ceil(n_loc / (2 * rows_pp)) * (2 * rows_pp))
    npair = n_loc_pad // rows_pp
    nt3 = n_loc_pad // npt2
    pb = _chunk_pair_bounds(npair)
    rbounds = np.array([b * rows_pp for b in pb], np.int64)

    # chunk-major table position of global node j = c*n_loc + r: chunk q's
    # region starts at n_cores*rbounds[q]; core c's stripe inside it.
    def tab_pos(j):
        c, r = np.divmod(j, n_loc)
        q = np.searchsorted(rbounds, r, side="right") - 1
        crk = rbounds[q + 1] - rbounds[q]
        return n_cores * rbounds[q] + c * crk + (r - rbounds[q])

    n_tab = n_cores * n_loc_pad

    w = weights
    u1 = (pos @ (w['w1a'][:3] + w['w1a'][3:6])).astype(np.float32)
    u1_full = np.zeros((n_tab, 64), np.float32)
    u1_full[tab_pos(np.arange(n_nodes))] = u1
    slot_pos = tab_pos(slot_src)  # [n_nodes, d_grp] chunk-major positions

    sel = np.zeros((128, 1024), np.float32)
    jj = np.arange(1024)
    sel[jj // d_grp, jj] = 1.0
    sel[64 + jj // d_grp, jj] = 1.0

    common = dict(
        u1_full=u1_full.astype(_BF),
        selAB=sel.astype(_BF),
        id128=np.eye(128, dtype=np.float32).astype(_BF),
        wbd1=_blockdiag2(w['w1b']).astype(_BF),
        wbd2=_blockdiag2(w['w2b']).astype(_BF),
        w3b=w['w3b'].astype(_BF),
        wst2=np.concatenate([w['w2a'][:64], w['w2a'][:64]], 0).astype(_BF),
        wst3=np.concatenate([w['w3a'][:64], w['w3a'][:64]], 0).astype(_BF),
        wpst2=_posw_stack(w['w2a'][64:67]),
        wpst3=_posw_stack(w['w3a'][64:67]),
        fbias=np.stack(
            [np.concatenate([w['b1b'], w['b1b']]),
             np.concatenate([w['b2b'], w['b2b']]),
             np.concatenate([w['b3b']])], axis=1).astype(np.float32),
    )

    per_core = []
    for cid in range(n_cores):
        lo = cid * n_loc
        pos_l = np.zeros((n_loc_pad, 3), np.float32)
        pos_l[:n_loc] = pos[lo:lo + n_loc]
        ss = np.zeros((n_loc_pad, d_grp), np.int64)
        ss[:n_loc] = slot_pos[lo:lo + n_loc]

        edges = ss.reshape(n_loc_pad // npt2, npt2 * d_grp)  # [tile, 1024]
        epair = edges.reshape(npair, 2, 1024)
        g12 = epair.reshape(npair, 2, 8, 128)
        g12 = g12.transpose(3, 0, 1, 2).reshape(128, npair * 16)
        gidx12 = np.ascontiguousarray(g12, dtype=np.int32)

        g3 = edges.reshape(nt3, 8, 128).transpose(2, 0, 1).reshape(128, nt3 * 8)
        gidx3 = np.ascontiguousarray(g3, dtype=np.int32)

        def vpack12(vloc):
            vv = vloc.reshape(npair, 2, npt2, 64)
            out = np.zeros((128, npair, 128), np.float32)
            out[0:npt2, :, 0:64] = vv[:, 0].transpose(1, 0, 2)
            out[64:64 + npt2, :, 64:128] = vv[:, 1].transpose(1, 0, 2)
            return np.ascontiguousarray(
                out.reshape(128, npair * 128)).astype(_BF)

        def vpack3(vloc):
            vv = vloc.reshape(nt3, npt2, 128)
            out = np.zeros((128, nt3 // 2, 128), np.float32)
            out[0:npt2, :, :] = vv[0::2].transpose(1, 0, 2)
            out[64:64 + npt2, :, :] = vv[1::2].transpose(1, 0, 2)
            return np.ascontiguousarray(
                out.reshape(128, (nt3 // 2) * 128)).astype(_BF)

        v1loc = w['b1a'] - pos_l @ w['w1a'][3:6]
        v2loc = w['b2a'] - pos_l @ w['w2a'][64:67]
        v3loc = w['b3a'] - pos_l @ w['w3a'][64:67]

        pp6 = np.zeros((35, npair * npt2), np.float32)
        pl = pos_l.reshape(npair, 2, npt2, 3)
        pp6[0:3] = pl[:, 0].transpose(2, 0, 1).reshape(3, npair * npt2)
        pp6[32:35] = pl[:, 1].transpose(2, 0, 1).reshape(3, npair * npt2)

        per_core.append(dict(
            gidx12=gidx12, gidx3=gidx3,
            vp1=vpack12(v1loc), vp2=vpack12(v2loc), vp3=vpack3(v3loc),
            posTp=pp6.astype(_BF)))

    cfg = dict(n_nodes=n_nodes, n_loc_pad=n_loc_pad, d_grp=d_grp,
               n_cores=n_cores)
    meta = dict(n_loc=n_loc, deg0=deg0)
    return cfg, common, per_core, meta


def kernel(pos, edge_index, batch, timestep,
           w1a, b1a, w1b, b1b, w2a, b2a, w2b, b2b,
           w3a, b3a, w3b, b3b, wr1, br1, wr2, br2):
    from concourse import bass_utils

    pos = np.asarray(pos, np.float32)
    edge_index = np.asarray(edge_index, np.int32)
    batch = np.asarray(batch, np.int32)
    W = {k: np.asarray(v, np.float32) for k, v in dict(
        w1a=w1a, b1a=b1a, w1b=w1b, b1b=b1b, w2a=w2a, b2a=b2a, w2b=w2b,
        b2b=b2b, w3a=w3a, b3a=b3a, w3b=w3b, b3b=b3b).items()}

    n_cores = NCORES
    cfg, common, per_core, meta = _prep(pos, edge_index, W, n_cores)
    key = tuple(sorted(cfg.items()))
    if key not in _CACHE:
        _CACHE[key] = _build_nc(**cfg)
    nc = _CACHE[key]

    in_maps = [dict(common, **per_core[c]) for c in range(n_cores)]
    res = bass_utils.run_bass_kernel_spmd(
        nc, in_maps, core_ids=list(range(n_cores)))

    n_loc = meta["n_loc"]
    h3 = np.concatenate(
        [np.asarray(res.results[c]["hT3"])[:, :n_loc].T
         for c in range(n_cores)], 0).astype(np.float32)
    if meta["deg0"] is not None and meta["deg0"].any():
        h3[meta["deg0"]] = 0.0

    nb = 64 if pos.shape[0] == N else int(batch.max()) + 1
    sums = np.zeros((nb, 128), np.float64)
    np.add.at(sums, batch, h3.astype(np.float64))
    counts = np.bincount(batch, minlength=nb).astype(np.float64)
    pooled = (sums / np.maximum(counts, 1.0)[:, None]).astype(np.float32)
    out = pooled @ np.asarray(wr1, np.float32) + np.asarray(br1, np.float32)
    out = out @ np.asarray(wr2, np.float32) + np.asarray(br2, np.float32)
    out = 1.0 / (1.0 + np.exp(-out))
    return out.squeeze(-1).astype(np.float32)
